# revision 16
# baseline (speedup 1.0000x reference)
"""Bass/Tile TRN2 kernel for nn_Attention_26388279067013.

Computes, for each batch row b:
    feat = enc @ We.T + dec @ Ws.T + cov[:,None] * Wc.sum(1) + b     [S, H]
    att  = tanh(feat) @ v_w                                          [S]
    att[s >= L_b] = -inf ; w = softmax(att) ; new_cov = cov + w
Returns (attention_weights [B,S], new_coverage [B,S]) both float32.

Sharding: data-parallel over B across 8 NeuronCores (4 rows each).

The enc matmul stream runs in bf16 (inputs quantized host-side): the
PE sustains 1 col/cycle with 2-byte moving operands. The rank-2 term
(bias+dec row, cov*wc outer product) is load-balanced between the PE
and the DVE: for 5 of every 8 s-tiles it is a 5th K=128 zero-padded
matmul; for the other 3 the DVE pre-seeds the PSUM tile
((wc_bcast * cov_col) + aug_bcast) and the enc matmuls accumulate on
top with start=False. This equalizes PE (~128us) and DVE
(~124us incl. the v-dot) busy time. All per-batch operands are
host-precomputed; the Scalar/ACT queue carries only tanh/exp (plus
one ek DMA per chunk); PSUM accumulation and softmax stay fp32.
"""

import sys

sys.path.insert(0, "/opt/trn_rl_repo")

import ml_dtypes
import numpy as np

import concourse.bacc as bacc
import concourse.tile as tile
import concourse.mybir as mybir
from concourse.bass_utils import run_bass_kernel_spmd
import bass_rust

B, S, H, D = 32, 4096, 512, 256
N_CORES = 8
B_LOC = B // N_CORES          # 4 batch rows per core
F32 = mybir.dt.float32
BF16 = mybir.dt.bfloat16
ALU = mybir.AluOpType
ACTF = mybir.ActivationFunctionType
BF16_NP = ml_dtypes.bfloat16
FP8 = mybir.dt.float8e4
FP8_NP = ml_dtypes.float8_e4m3

N_K = H // 128                # 4 contraction tiles
N_CHUNK = S // 1024           # 4 dma chunks per batch (2KB/partition lines)
NEG_BIG = -30000.0            # exp(x - 30000) == 0.0 exactly in f32
N_WARM = 8


def build_kernel():
    nc = bacc.Bacc("TRN2", debug=False, num_devices=N_CORES)

    # ---- dram I/O (per core) ----
    encT = nc.dram_tensor("encT", [B_LOC, 128, N_K, S], FP8, kind="ExternalInput").ap()
    cov = nc.dram_tensor("cov", [B_LOC, 32, 128], F32, kind="ExternalInput").ap()
    cov16 = nc.dram_tensor("cov16", [B_LOC, 1, S], BF16, kind="ExternalInput").ap()
    aug2 = nc.dram_tensor("aug2", [B_LOC, 2, H], BF16, kind="ExternalInput").ap()
    lens_col = nc.dram_tensor("lens_col", [B_LOC, 128, 1], F32, kind="ExternalInput").ap()
    WeT = nc.dram_tensor("WeT", [128, N_K, H], FP8, kind="ExternalInput").ap()
    v_bc_d = nc.dram_tensor("v_bc", [128, H], BF16, kind="ExternalInput").ap()
    iota_d = nc.dram_tensor("iota_pm", [128, 32], F32, kind="ExternalInput").ap()
    ident_d = nc.dram_tensor("ident", [128, 128], F32, kind="ExternalInput").ap()
    ones_row = nc.dram_tensor("ones_row", [1, S], BF16, kind="ExternalInput").ap()
    zeros_d = nc.dram_tensor("zeros_pad", [126, S], BF16, kind="ExternalInput").ap()
    out_w = nc.dram_tensor("out_w", [B_LOC, 32, 128], F32, kind="ExternalOutput").ap()
    out_c = nc.dram_tensor("out_c", [B_LOC, 32, 128], F32, kind="ExternalOutput").ap()

    with tile.TileContext(nc) as tc:
        with (
            tc.tile_pool(name="persist", bufs=1) as pp,
            tc.tile_pool(name="enc", bufs=10) as encp,
            tc.tile_pool(name="x", bufs=4) as xp,
            tc.tile_pool(name="scratch", bufs=2) as scrp,
            tc.tile_pool(name="small", bufs=4) as smp,
            tc.tile_pool(name="batch", bufs=3) as bp,
            tc.tile_pool(name="psum", bufs=3, space="PSUM") as psp,
            tc.tile_pool(name="psum_misc", bufs=2, space="PSUM") as psm,
        ):
            # ---- one-time setup ----
            # warm memset first on the vector queue so the PE warmup burst
            # starts right after the framework preamble.
            warm = pp.tile([128, 512], BF16, tag="warm")
            nc.vector.memset(warm[:], 0.5)
            for wi in range(N_WARM):
                ps_w = psm.tile([128, 512], F32, tag="mpsum")
                nc.tensor.matmul(ps_w[:], warm[:, 0:128], warm[:],
                                 start=True, stop=True)
            ones_k1 = pp.tile([1, 128], F32, tag="ones_k1")
            nc.vector.memset(ones_k1[:], 1.0)
            ones_col = pp.tile([128, 1], F32, tag="ones_col")
            nc.vector.memset(ones_col[:], 1.0)

            # 3-way ring of per-batch rank-2 PE operands (written by
            # prep(b), read by heavy(b); 3 deep so prep(b+2) never
            # clobbers live data). Slot-0 memsets first so batch 0 is
            # ready the moment its DMAs land.
            cov_pad = [None] * 3
            aug128 = [None] * 3
            for par in range(3):
                cp = pp.tile([128, S], BF16, tag=f"cov_pad{par}")
                ag = pp.tile([128, H], BF16, tag=f"aug128_{par}")
                if par == 0:
                    # slot 0 is needed ~11us in: cheap DVE memsets
                    nc.vector.memset(cp[:], 0.0)
                    nc.vector.memset(ag[:], 0.0)
                else:
                    # slots 1-2 are needed tens of us later: zero them via
                    # DMA so the DVE spends no time on them
                    nc.gpsimd.dma_start(cp[2:128, :], zeros_d[:, :])
                    nc.gpsimd.dma_start(cp[0:2, :], zeros_d[0:2, :])
                    nc.gpsimd.dma_start(ag[0:126, :], zeros_d[:, 0:H])
                    nc.gpsimd.dma_start(ag[126:128, :], zeros_d[0:2, 0:H])
                cov_pad[par] = cp
                aug128[par] = ag

            # gpsimd queue: weights first (first matmul needs them), the
            # first enc chunk rides sync in parallel.
            wet_dr = pp.tile([128, N_K, H], FP8, tag="wet_dr")
            nc.gpsimd.dma_start(wet_dr[:], WeT[:, :, :])
            v_bcast = pp.tile([128, H], BF16, tag="v_bcast")
            nc.gpsimd.dma_start(v_bcast[:], v_bc_d[:, :])
            iota_sb = pp.tile([128, 32], F32, tag="iota")
            nc.gpsimd.dma_start(iota_sb[:], iota_d[:, :])
            ident_sb = pp.tile([128, 128], F32, tag="ident")
            nc.gpsimd.dma_start(ident_sb[:], ident_d[:, :])

            # ---- per batch, software-pipelined ----
            state = {}

            def emit_prep(b):
                par = b % 3
                nc.sync.dma_start(cov_pad[par][0:1, :], ones_row[:, :])
                nc.sync.dma_start(cov_pad[par][1:2, :], cov16[b, :, :])
                nc.gpsimd.dma_start(aug128[par][0:2, :], aug2[b, :, :])
                l_col = smp.tile([128, 1], F32, tag="l_col")
                nc.gpsimd.dma_start(l_col[:], lens_col[b, :, :])
                att_pm = bp.tile([128, 32], F32, tag="att_pm")
                state[b] = dict(l_col=l_col, att_pm=att_pm, par=par)

            def emit_heavy_chunk(b, c):
                st8 = state[b]
                par = st8["par"]
                ek = encp.tile([128, N_K, 1024], FP8, tag="enc")
                src = encT[b, :, :, c * 1024:(c + 1) * 1024]
                nc.sync.dma_start(ek[:, 0:2, :], src[:, 0:2, :])
                nc.sync.dma_start(ek[:, 2:4, :], src[:, 2:4, :])
                for t2 in range(4):
                    ps = psp.tile([128, 1024], F32, tag="feat")
                    for half in range(2):
                        lh = 2 * t2 + half
                        st = 8 * c + lh
                        scol = lh * 128
                        dst = ps[:, half * 512:(half + 1) * 512]
                        # DoubleRow fp8: two K=256 passes at 0.5 cyc/col.
                        for j in (0, 2):
                            nc.tensor.matmul(
                                dst, ek[:, j:j + 2, scol:scol + 128],
                                wet_dr[:, j:j + 2, :],
                                start=(j == 0), stop=False,
                                perf_mode=mybir.MatmulPerfMode.DoubleRow)
                        nc.tensor.matmul(
                            dst, cov_pad[par][:, st * 128:(st + 1) * 128],
                            aug128[par][:], start=False, stop=True)
                        # Dep-free filler matmul: the whole core shares one
                        # clock domain, and with DR the PE would otherwise
                        # idle enough for the HAM to downclock (k=8 -> k=4),
                        # slowing the critical-path DVE/ACT by ~20%. Keeping
                        # the PE busy holds k=8/8.
                        ps_w = psm.tile([128, 512], F32, tag="mpsum")
                        nc.tensor.matmul(ps_w[:], warm[:, 0:128], warm[:],
                                         start=True, stop=True)
                    x = xp.tile([128, 1024], BF16, tag="x")
                    nc.scalar.activation(x[:], ps[:], ACTF.Tanh)
                    for half in range(2):
                        st = 8 * c + 2 * t2 + half
                        scr = scrp.tile([128, 512], BF16, tag="vscr")
                        nc.vector.scalar_tensor_tensor(
                            scr[:], x[:, half * 512:(half + 1) * 512],
                            1.0, v_bcast[:], ALU.bypass, ALU.mult,
                            accum_out=st8["att_pm"][:, st:st + 1])

            def emit_softmax_a(b):
                # Phase a: mask + exp + PE transpose + Pool all-reduce of
                # the denominator. Emitted mid-stream; nothing here makes
                # the DVE wait on a queued PE/ACT op.
                st8 = state[b]
                att_pm, l_col = st8["att_pm"], st8["l_col"]
                pad01 = bp.tile([128, 32], F32, tag="pad01")
                nc.vector.tensor_scalar(pad01[:], iota_sb[:], l_col[:], None, ALU.is_ge)
                att_m = bp.tile([128, 32], F32, tag="att_m")
                nc.vector.scalar_tensor_tensor(
                    att_m[:], pad01[:], NEG_BIG, att_pm[:], ALU.mult, ALU.add)
                exp_pm = bp.tile([128, 32], F32, tag="exp_pm")
                rowsum = smp.tile([128, 1], F32, tag="rowsum")
                nc.scalar.activation(exp_pm[:], att_m[:], ACTF.Exp, accum_out=rowsum[:])
                covT = bp.tile([32, 128], F32, tag="covT")
                nc.sync.dma_start(covT[:], cov[b, :, :])
                st8["covT"] = covT
                ps_t = psm.tile([32, 128], F32, tag="mpsum")
                nc.tensor.transpose(ps_t[:], exp_pm[:], ident_sb[:])
                denom = smp.tile([128, 1], F32, tag="denom")
                nc.gpsimd.partition_all_reduce(denom[:], rowsum[:], 128,
                                               bass_rust.ReduceOp.add)
                st8["ps_t"] = ps_t
                st8["denom"] = denom

            def emit_softmax_b(b):
                # Phase b, ~5us later: everything it reads (transpose,
                # all-reduced denominator) has long drained, so the
                # in-order DVE queue never head-of-line blocks.
                st8 = state.pop(b)
                ps_t, denom, covT = st8["ps_t"], st8["denom"], st8["covT"]
                rinv32 = smp.tile([32, 1], F32, tag="rinv32")
                nc.vector.reciprocal(rinv32[:], denom[0:32, :])
                w_sb = bp.tile([32, 128], F32, tag="w_sb")
                nc.vector.tensor_scalar(w_sb[:], ps_t[:], rinv32[:], None, ALU.mult)
                nc.sync.dma_start(out_w[b, :, :], w_sb[:])
                ncov = bp.tile([32, 128], F32, tag="ncov")
                nc.vector.scalar_tensor_tensor(
                    ncov[:], ps_t[:], rinv32[:], covT[:], ALU.mult, ALU.add)
                nc.sync.dma_start(out_c[b, :, :], ncov[:])

            emit_prep(0)
            emit_prep(1)
            for b in range(B_LOC):
                for c in range(N_CHUNK):
                    emit_heavy_chunk(b, c)
                    if c == 1 and b >= 1:
                        emit_softmax_a(b - 1)
                    if c == 2 and b + 2 < B_LOC:
                        emit_prep(b + 2)
                    if c == 3 and b >= 1:
                        emit_softmax_b(b - 1)
            emit_softmax_a(B_LOC - 1)
            emit_softmax_b(B_LOC - 1)

    nc.compile()
    return nc


_NC_CACHE = {}


def _get_nc():
    if "nc" not in _NC_CACHE:
        _NC_CACHE["nc"] = build_kernel()
    return _NC_CACHE["nc"]


def make_in_maps(dec_input, enc_output, coverage_vector, text_lengths, W, b, v_w, v_b):
    dec_input = np.asarray(dec_input, np.float32)
    enc_output = np.asarray(enc_output, np.float32)
    coverage_vector = np.asarray(coverage_vector, np.float32)
    lens_f = np.asarray(text_lengths).astype(np.float32)
    W = np.asarray(W, np.float32)
    b = np.asarray(b, np.float32)
    v_w = np.asarray(v_w, np.float32)

    # DoubleRow layout: [128 partitions, k_subtile, out] in e4m3
    WeT_dr = np.ascontiguousarray(
        W[:, :H].T.reshape(N_K, 128, H).transpose(1, 0, 2).astype(FP8_NP))
    Ws = W[:, H:H + D]                                            # [H, D]
    wc = W[:, H + D:].sum(axis=1).astype(np.float32)              # [H]
    aug_rows = dec_input[:, 0, :] @ Ws.T + b[None, :]             # [B, H] f32
    v_bc = np.broadcast_to(v_w[None, :], (128, H)).astype(BF16_NP)
    iota_pm = (np.arange(32)[None, :] * 128 + np.arange(128)[:, None]).astype(np.float32)
    ident = np.eye(128, dtype=np.float32)

    in_maps = []
    for core in range(N_CORES):
        lo = core * B_LOC
        hi = lo + B_LOC
        encT = np.ascontiguousarray(
            enc_output[lo:hi].transpose(0, 2, 1)                   # [B_LOC, H, S]
            .reshape(B_LOC, N_K, 128, S).transpose(0, 2, 1, 3)     # [B_LOC, 128, K, S]
            .astype(FP8_NP))
        covc = np.ascontiguousarray(coverage_vector[lo:hi].reshape(B_LOC, 32, 128))
        cov16c = np.ascontiguousarray(
            coverage_vector[lo:hi].reshape(B_LOC, 1, S).astype(BF16_NP))
        aug2c = np.ascontiguousarray(
            np.stack([aug_rows[lo:hi], np.broadcast_to(wc, (B_LOC, H))], axis=1)
            .astype(BF16_NP))                                      # [B_LOC, 2, H]
        lens_c = np.ascontiguousarray(
            np.broadcast_to(lens_f[lo:hi, None, None], (B_LOC, 128, 1))).copy()
        in_maps.append({
            "encT": encT,
            "cov": covc,
            "cov16": cov16c,
            "aug2": aug2c,
            "lens_col": lens_c,
            "WeT": WeT_dr, "v_bc": np.ascontiguousarray(v_bc),
            "iota_pm": iota_pm, "ident": ident,
            "ones_row": np.ones((1, S), BF16_NP),
            "zeros_pad": np.zeros((126, S), BF16_NP),
        })
    return in_maps


def kernel(dec_input, enc_output, coverage_vector, text_lengths, W, b, v_w, v_b,
           _trace=False):
    nc = _get_nc()
    in_maps = make_in_maps(dec_input, enc_output, coverage_vector, text_lengths,
                           W, b, v_w, v_b)
    res = run_bass_kernel_spmd(nc, in_maps, list(range(N_CORES)), trace=_trace)
    w = np.concatenate([r["out_w"].reshape(B_LOC, S) for r in res.results], axis=0)
    c = np.concatenate([r["out_c"].reshape(B_LOC, S) for r in res.results], axis=0)
    if _trace:
        kernel.last_result = res
    return w, c
